# revision 13
# baseline (speedup 1.0000x reference)
"""Trainium2 Bass kernel for nn_Attention_35107062677619.

Dense transformer attention block (B=2, S=2048, D=4096, 32 Q heads / 8 KV
heads, head_dim 128, RoPE, causal mask) tensor-parallel over 8 NeuronCores.

Sharding: each core owns 4 Q heads + their shared KV head (GQA groups align
with cores), computes projections + RoPE + attention for those heads, then an
on-device AllGather collects the per-core attention outputs and each core
applies its 512-row slice of wo.  The host concatenates the 8 output-feature
slices.

v3 structure: dense phase A (QKV projection + RoPE, PE-saturated stream with
6 PSUM banks), then a fused B+C phase where each (batch, q-tile)'s attention
heads are interleaved with wo-projection output-tiles of the slab gathered
two iterations earlier, so wo matmuls fill the softmax dependency bubbles.

Key optimizations over the original baseline:
 - causal trimming at 128-column granularity: diagonal k-tiles compute only
   q >= k columns; one shared [128,128] triangular exp-mask.
 - no ones-matmul: softmax denominator accumulated off-PE by DVE/gpsimd
   tensor_adds, cross-partition summed via gpsimd.partition_all_reduce, and
   inverted with the single-op reciprocal_approx_fast (the full-precision
   DVE reciprocal costs ~3.3us per call).
 - V transposed to [tok, hd] via DMA-crossbar transposes (no PSUM/PE).
 - AllGather outputs in Shared address space (fast HBM-HBM collective path);
   all large DMAs split into 512KB-or-less chunks to spread across queues.
"""

import math
import os

import numpy as np
import ml_dtypes

B = 2
S = 2048
D = 4096
HD = 128
N_HEADS = 32
N_KV = 8
N_CORES = 8
NQH = N_HEADS // N_CORES  # 4 local Q heads
P = 128
SLAB = 512  # token tile (matmul free dim)
KH = D // P  # 32 hidden k-tiles
QKVD = NQH * HD + 2 * HD  # 768 projection output dims
F32 = np.float32
BF16 = ml_dtypes.bfloat16


def _build(nc_cores=N_CORES, s=S):
    """Build the SPMD Bass program (one program, data-parallel over cores)."""
    import concourse.mybir as mybir
    import concourse.tile as tile
    from concourse import bacc, bass_isa

    f32 = mybir.dt.float32
    bf16 = mybir.dt.bfloat16
    EXP = mybir.ActivationFunctionType.Exp

    tok = B * s
    nslab = tok // SLAB  # 8
    sslab = s // SLAB  # 4 slabs per batch
    nkt = s // P  # 16 k-tiles of 128 per batch
    spk = SLAB // P  # 4
    nakt = (nc_cores * NQH * HD) // P  # 32 gathered k-tiles for wo
    C_LAG = 2

    nc = bacc.Bacc("TRN2", target_bir_lowering=False, debug=False,
                   num_devices=nc_cores)

    # x blocks laid out slab-major: [slab, kb, p, t]
    xT = nc.dram_tensor("xT", [nslab * KH * P, SLAB], bf16,
                        kind="ExternalInput")
    wqkvT = nc.dram_tensor("wqkvT", [D, QKVD], bf16, kind="ExternalInput")
    woT = nc.dram_tensor("woT", [nc_cores * NQH * HD, SLAB], bf16,
                         kind="ExternalInput")
    cosq = nc.dram_tensor("cosq", [P, s], bf16, kind="ExternalInput")
    sinq = nc.dram_tensor("sinq", [P, s], bf16, kind="ExternalInput")
    emaskd = nc.dram_tensor("emaskd", [P, P], bf16, kind="ExternalInput")
    outT = nc.dram_tensor("outT", [SLAB, tok], f32, kind="ExternalOutput")

    xT_v = xT.ap().rearrange("(sl k p) t -> sl k p t", sl=nslab, k=KH, p=P)
    wqkvT_r = wqkvT.ap().rearrange("(o p) q -> p o q", p=P)
    woT_r = woT.ap().rearrange("(o p) q -> p o q", p=P)

    with tile.TileContext(nc) as tc:
        with (
            tc.tile_pool(name="persist", bufs=1) as persist,
            tc.tile_pool(name="dram", bufs=1, space="DRAM") as dram,
        ):
            cc_in = [dram.tile([NQH * HD, SLAB], bf16, tag=f"cc_in{i}",
                               name=f"cc_in{i}")
                     for i in range(nslab)]
            cc_out = [dram.tile([nc_cores * NQH * HD, SLAB], bf16,
                                tag=f"cc_out{i}", name=f"cc_out{i}",
                                addr_space="Shared")
                      for i in range(nslab)]
            cc_out_r = [t[:].rearrange("(o p) t -> p o t", p=P)
                        for t in cc_out]

            emask_sb = persist.tile([P, P], bf16, tag="emaskd")
            nc.sync.dma_start(emask_sb[:], emaskd.ap())
            QTa = persist.tile([P, NQH, tok], bf16, tag="QTa")
            KT = persist.tile([P, tok], bf16, tag="KT")
            V = persist.tile([P, B * nkt, HD], bf16, tag="V")
            cos_sb = persist.tile([P, s], bf16, tag="cos")
            sin_sb = persist.tile([P, s], bf16, tag="sin")
            nc.sync.dma_start(cos_sb[:], cosq.ap())
            nc.sync.dma_start(sin_sb[:], sinq.ap())

            # ---- Phase A: dense QKV projection + RoPE ----
            with (
                tc.tile_pool(name="wqkvp", bufs=1) as wpool,
                tc.tile_pool(name="xa", bufs=8) as xpool,
                tc.tile_pool(name="rp", bufs=3) as rp,
                tc.tile_pool(name="psA", bufs=6, space="PSUM") as psA,
            ):
                wqkv_sb = wpool.tile([P, KH, QKVD], bf16, tag="wqkv")
                for c in range(8):
                    nc.sync.dma_start(wqkv_sb[:, c * 4:(c + 1) * 4, :],
                                      wqkvT_r[:, c * 4:(c + 1) * 4, :])

                def emit_rope(ps, dst, cs_sl, sn_sl, alt, nm):
                    h = P // 2
                    q_sb = rp.tile([P, SLAB], bf16, tag="qsb",
                                   name=f"qsb_{nm}")
                    if alt:
                        nc.scalar.copy(q_sb[:], ps[:])
                    else:
                        nc.vector.tensor_copy(q_sb[:], ps[:])
                    tmp = rp.tile([P, SLAB], bf16, tag="rtmp",
                                  name=f"rt_{nm}")
                    nc.vector.tensor_copy(tmp[0:h, :], q_sb[h:P, :])
                    nc.vector.tensor_copy(tmp[h:P, :], q_sb[0:h, :])
                    nc.vector.tensor_mul(tmp[:], tmp[:], sn_sl)
                    nc.vector.tensor_mul(dst, q_sb[:], cs_sl)
                    nc.vector.tensor_add(dst, dst, tmp[:])

                for slab in range(nslab):
                    b, qt = divmod(slab, sslab)
                    t0 = slab * SLAB
                    sr = qt * SLAB
                    nm = f"{b}_{qt}"
                    cs_sl = cos_sb[:, sr:sr + SLAB]
                    sn_sl = sin_sb[:, sr:sr + SLAB]
                    psums = [psA.tile([P, SLAB], f32, tag="proj",
                                      name=f"pj_{nm}_{d}")
                             for d in range(6)]
                    for kb in range(KH):
                        xt = xpool.tile([P, SLAB], bf16, tag="x",
                                        name=f"x_{nm}_{kb}")
                        nc.sync.dma_start(xt[:], xT_v[slab, kb])
                        for d in range(6):
                            nc.tensor.matmul(
                                psums[d][:],
                                wqkv_sb[:, kb, d * P:(d + 1) * P],
                                xt[:],
                                start=(kb == 0), stop=(kb == KH - 1))
                    for d in range(NQH):
                        emit_rope(psums[d], QTa[:, d, t0:t0 + SLAB],
                                  cs_sl, sn_sl, d % 2 == 1, f"{nm}_q{d}")
                    emit_rope(psums[NQH], KT[:, t0:t0 + SLAB],
                              cs_sl, sn_sl, True, f"{nm}_k")
                    vtmp = rp.tile([P, SLAB], bf16, tag="vtmp",
                                   name=f"vt_{nm}")
                    nc.vector.tensor_copy(vtmp[:], psums[NQH + 1][:])
                    for jj in range(spk):
                        nc.sync.dma_start(
                            V[:, b * nkt + qt * spk + jj, :],
                            vtmp[:, jj * P:(jj + 1) * P],
                            transpose=True)

            # ---- Phase B+C: attention interleaved with wo projection ----
            with (
                tc.tile_pool(name="wop", bufs=1) as wop,
                tc.tile_pool(name="gp", bufs=2) as gp,
                tc.tile_pool(name="esp", bufs=6) as esp,
                tc.tile_pool(name="accp", bufs=2) as accp,
                tc.tile_pool(name="op", bufs=2) as op,
                tc.tile_pool(name="ocp", bufs=3) as ocp,
                tc.tile_pool(name="psS", bufs=3, space="PSUM") as psS,
                tc.tile_pool(name="psAV", bufs=2, space="PSUM") as psAV,
                tc.tile_pool(name="psC", bufs=3, space="PSUM") as psC,
            ):
                wo_sb = wop.tile([P, nakt, SLAB], bf16, tag="wo")
                for c in range(8):
                    nc.sync.dma_start(wo_sb[:, c * 4:(c + 1) * 4, :],
                                      woT_r[:, c * 4:(c + 1) * 4, :])

                gtiles = {}

                def emit_g_load(cs):
                    g = gp.tile([P, nakt, SLAB], bf16, tag="g",
                                name=f"g_{cs}")
                    for c in range(8):
                        nc.sync.dma_start(
                            g[:, c * 4:(c + 1) * 4, :],
                            cc_out_r[cs][:, c * 4:(c + 1) * 4, :])
                    gtiles[cs] = g

                def emit_C_od(cs, od):
                    g = gtiles[cs]
                    ps = psC.tile([P, SLAB], f32, tag="wops",
                                  name=f"wops_{cs}_{od}")
                    for kb in range(nakt):
                        nc.tensor.matmul(
                            ps[:], wo_sb[:, kb, od * P:(od + 1) * P],
                            g[:, kb, :],
                            start=(kb == 0), stop=(kb == nakt - 1))
                    oc = ocp.tile([P, SLAB], f32, tag="oc",
                                  name=f"oc_{cs}_{od}")
                    if od % 2:
                        nc.scalar.copy(oc[:], ps[:])
                    else:
                        nc.vector.tensor_copy(oc[:], ps[:])
                    nc.sync.dma_start(
                        outT.ap()[od * P:(od + 1) * P,
                                  cs * SLAB:(cs + 1) * SLAB], oc[:])

                def emit_head(b, qt, l, slab):
                    nkb = spk * (qt + 1)
                    pfx = f"{b}_{qt}_{l}"
                    acc_d = accp.tile([P, SLAB], f32, tag="accd",
                                      name=f"accd_{pfx}")
                    acc_p = accp.tile([P, SLAB], f32, tag="accp",
                                      name=f"accp_{pfx}")
                    nc.vector.memset(acc_d[:], 0.0)
                    nc.gpsimd.memset(acc_p[:], 0.0)
                    av = psAV.tile([P, SLAB], f32, tag="av",
                                   name=f"av_{pfx}")
                    for kb in range(nkb):
                        j = kb - (nkb - spk)
                        qoff = j * P if j > 0 else 0
                        w = SLAB - qoff
                        stg = psS.tile([P, SLAB], f32, tag="st",
                                       name=f"st_{pfx}_{kb}")
                        nc.tensor.matmul(
                            stg[:, 0:w],
                            KT[:, b * s + kb * P:b * s + (kb + 1) * P],
                            QTa[:, l, slab * SLAB + qoff:
                                (slab + 1) * SLAB],
                            start=True, stop=True)
                        es = esp.tile([P, SLAB], bf16, tag="es",
                                      name=f"es_{pfx}_{kb}")
                        nc.scalar.activation(es[:, 0:w], stg[:, 0:w], EXP)
                        if j >= 0:
                            nc.vector.tensor_mul(es[:, 0:P], es[:, 0:P],
                                                 emask_sb[:])
                        if kb % 3 != 2:
                            nc.vector.tensor_add(acc_d[:, qoff:SLAB],
                                                 acc_d[:, qoff:SLAB],
                                                 es[:, 0:w])
                        else:
                            nc.gpsimd.tensor_add(acc_p[:, qoff:SLAB],
                                                 acc_p[:, qoff:SLAB],
                                                 es[:, 0:w])
                        nc.tensor.matmul(
                            av[:, qoff:SLAB], V[:, b * nkt + kb, :],
                            es[:, 0:w],
                            start=(kb == 0), stop=(kb == nkb - 1),
                            skip_group_check=True)
                    o_u = op.tile([P, SLAB], bf16, tag="ou",
                                  name=f"ou_{pfx}")
                    if l % 2:
                        nc.scalar.copy(o_u[:], av[:])
                    else:
                        nc.vector.tensor_copy(o_u[:], av[:])
                    nc.vector.tensor_add(acc_d[:], acc_d[:], acc_p[:])
                    accr = accp.tile([P, SLAB], f32, tag="accr",
                                     name=f"accr_{pfx}", bufs=1)
                    nc.gpsimd.partition_all_reduce(
                        accr[:], acc_d[:], channels=P,
                        reduce_op=bass_isa.ReduceOp.add)
                    nc.vector.reciprocal_approx_fast(acc_p[:], accr[:])
                    o = op.tile([P, SLAB], bf16, tag="o", name=f"o_{pfx}")
                    nc.vector.tensor_mul(o[:], o_u[:], acc_p[:])
                    nc.sync.dma_start(cc_in[slab][l * HD:(l + 1) * HD, :],
                                      o[:])

                for slab in range(nslab):
                    b, qt = divmod(slab, sslab)
                    if slab >= 1:
                        emit_g_load(slab - 1)
                    for l in range(NQH):
                        emit_head(b, qt, l, slab)
                        if slab >= C_LAG:
                            emit_C_od(slab - C_LAG, l)
                    nc.gpsimd.collective_compute(
                        "AllGather",
                        mybir.AluOpType.bypass,
                        ins=[cc_in[slab].opt()],
                        outs=[cc_out[slab].opt()],
                        replica_groups=[list(range(nc_cores))],
                    )
                emit_g_load(nslab - 1)
                for cs in range(nslab - C_LAG, nslab):
                    for od in range(spk):
                        emit_C_od(cs, od)

    nc.compile()
    return nc


def _prep_inputs(x, wq, wk, wv, wo, freqs_cos, freqs_sin, mask,
                 nc_cores=N_CORES, s=S):
    """Host-side sharding + layout prep. Returns per-core input maps."""
    tok = B * s
    x = np.asarray(x, F32)
    nslab = tok // SLAB
    # slab-major tiled layout: [slab, kb, p, t]
    xT = np.ascontiguousarray(
        x.reshape(nslab, SLAB, D // P, P).transpose(0, 2, 3, 1)
    ).astype(BF16).reshape(nslab * D // P * P, SLAB)

    # de-interleave permutation within a head: [x0_0..x0_63, x1_0..x1_63]
    perm = np.concatenate([np.arange(0, HD, 2), np.arange(1, HD, 2)])

    cos = np.asarray(freqs_cos, F32)  # [s, 64]
    sin = np.asarray(freqs_sin, F32)
    cosq = np.ascontiguousarray(
        np.concatenate([cos.T, cos.T], axis=0)).astype(BF16)
    # the shifted partner is multiplied by the DESTINATION row's sin entry:
    # o_top = x0*c - x1*s  -> top rows carry -sin
    # o_bot = x1*c + x0*s  -> bottom rows carry +sin
    sinq = np.ascontiguousarray(
        np.concatenate([-sin.T, sin.T], axis=0)).astype(BF16)

    # one shared [k, q] lower-triangular (incl diag) 0/1 mask for the
    # 128x128 diagonal blocks
    emaskd = np.ascontiguousarray(
        np.tril(np.ones((P, P), dtype=F32)).T).astype(BF16)

    scale = 1.0 / math.sqrt(HD)
    in_maps = []
    for c in range(nc_cores):
        wq_c = np.asarray(wq, F32)[c * NQH * HD:(c + 1) * NQH * HD]  # [512, D]
        wq_c = (wq_c.reshape(NQH, HD, D)[:, perm, :] * scale).reshape(
            NQH * HD, D)
        wk_c = np.asarray(wk, F32)[c * HD:(c + 1) * HD][perm, :]  # [128, D]
        wv_c = np.asarray(wv, F32)[c * HD:(c + 1) * HD]  # [128, D]
        wqkvT = np.ascontiguousarray(
            np.concatenate([wq_c, wk_c, wv_c], axis=0).T).astype(BF16)
        woT = np.ascontiguousarray(
            np.asarray(wo, F32)[c * SLAB:(c + 1) * SLAB].T).astype(BF16)
        in_maps.append({
            "xT": xT,
            "wqkvT": wqkvT,
            "woT": woT,
            "cosq": cosq,
            "sinq": sinq,
            "emaskd": emaskd,
        })
    return in_maps


_NC_CACHE = {}


def _get_nc(nc_cores=N_CORES, s=S):
    key = (nc_cores, s)
    if key not in _NC_CACHE:
        _NC_CACHE[key] = _build(nc_cores, s)
    return _NC_CACHE[key]


def _assemble(results, nc_cores=N_CORES, s=S):
    out = np.empty((B, s, nc_cores * SLAB), dtype=F32)
    for c in range(nc_cores):
        oT = results[c]["outT"]  # [512, tok]
        out[:, :, c * SLAB:(c + 1) * SLAB] = oT.T.reshape(B, s, SLAB)
    return out


def _run(inputs, trace=False, nc_cores=N_CORES, s=S):
    from concourse.bass_utils import run_bass_kernel_spmd

    nc = _get_nc(nc_cores, s)
    in_maps = _prep_inputs(**inputs, nc_cores=nc_cores, s=s)
    res = run_bass_kernel_spmd(nc, in_maps, core_ids=list(range(nc_cores)),
                               trace=trace)
    return _assemble(res.results, nc_cores, s), res


def kernel(x, wq, wk, wv, wo, freqs_cos, freqs_sin, mask):
    out, _ = _run(dict(x=x, wq=wq, wk=wk, wv=wv, wo=wo,
                       freqs_cos=freqs_cos, freqs_sin=freqs_sin, mask=mask),
                  trace=bool(int(os.environ.get("KERNEL_TRACE", "0"))))
    return out
